# revision 1
# baseline (speedup 1.0000x reference)
"""DNDT forward kernel for Trainium2 (8 NeuronCores, data-parallel).

Math (matches the reference):
    w = [1,2,3,4];  b = [0, cumsum(-sort(beta))]
    sigma[i,f,k] = sigmoid((x[i,f]*w[k] + b[k]) / T)            [B, 6, 4]
    leaves[i]    = kron(sigma[i,0], ..., sigma[i,5])            [B, 4096]
    out          = leaves @ L                                   [B, 10]

Restructured to avoid materializing the 4096-wide leaves:
    A[i,a]  = kron(s0, s1)          a = k0*4+k1      in [0,16)
    Bm[i,b] = kron(s2, s3, s4, s5)  b = k2*64+...+k5 in [0,256)
    M[i,(a,c)] = sum_b Bm[i,b] * L3[b, (a,c)]   (PE matmul, K=256 in 2 chunks)
        where L3[b, a*10+c] = L[a*256+b, c]
    out[i,c] = sum_a A[i,a] * M[i,(a,c)]        (DVE multiply + strided reduce)

Per-core layout: 8192 rows, processed as 8 supertiles of 1024 rows.
Within a supertile, partition p holds rows {base + p*G + q : q in [0,G)}.
For each q, Bm[:, q, :] is a [128,256] row-major block; two PE transposes
produce the [256,128] lhsT for the matmul.
"""

import numpy as np

import concourse.bacc as bacc
import concourse.mybir as mybir
import concourse.tile as tile
from concourse.bass_utils import run_bass_kernel_spmd

F32 = mybir.dt.float32
F16 = mybir.dt.float16

B, F, NB, NCLS = 65536, 6, 4, 10
CORES = 8
ROWS = B // CORES          # 8192 rows per core
G = 8                      # row-groups (matmul tiles) per supertile
ST_ROWS = 128 * G          # 1024 rows per supertile
N_ST = ROWS // ST_ROWS     # 8 supertiles
TEMP = 0.1

_NC_CACHE = {}

import os
ACC_MODE = os.environ.get("K_ACC", "fast")   # "fast" | "mid" | "acc"


def _build_nc():
    nc = bacc.Bacc("TRN2", target_bir_lowering=False, debug=False)

    xc = nc.dram_tensor("xc", [ROWS, F], F32, kind="ExternalInput")
    wt = nc.dram_tensor("wt", [128, 24], F32, kind="ExternalInput")
    bt = nc.dram_tensor("bt", [128, 24], F32, kind="ExternalInput")
    ident = nc.dram_tensor("ident", [128, 128], F16, kind="ExternalInput")
    l3p = nc.dram_tensor("l3p", [128, 2, 160], F16, kind="ExternalInput")
    outc = nc.dram_tensor("outc", [ROWS, NCLS], F32, kind="ExternalOutput")

    with tile.TileContext(nc) as tc:
        with (
            tc.tile_pool(name="consts", bufs=1) as consts,
            tc.tile_pool(name="io", bufs=4) as io,
            tc.tile_pool(name="work", bufs=3) as work,
            tc.tile_pool(name="wts", bufs=4) as wts,
            tc.tile_pool(name="ps_t", bufs=2, space="PSUM") as ps_t,
            tc.tile_pool(name="ps_m", bufs=3, space="PSUM") as ps_m,
        ):
            wt_sb = consts.tile([128, 24], F32)
            nc.sync.dma_start(wt_sb[:, :], wt[:, :])
            bt_sb = consts.tile([128, 24], F32)
            nc.sync.dma_start(bt_sb[:, :], bt[:, :])
            id_sb = consts.tile([128, 128], F16)
            nc.sync.dma_start(id_sb[:, :], ident[:, :])
            l3_sb = consts.tile([128, 2, 160], F16)
            nc.sync.dma_start(l3_sb[:, :, :], l3p[:, :, :])

            for st in range(N_ST):
                base = st * ST_ROWS
                # partition p <- rows base + p*G + q (q = 0..G-1), contiguous per partition
                xs = xc[base:base + ST_ROWS, :].rearrange("(p g) f -> p g f", g=G)
                x_sb = io.tile([128, G, F], F32, tag="x")
                nc.sync.dma_start(x_sb[:, :, :], xs)

                # z[p,g,f,k] = x[p,g,f] * (w[k]/T) + (b[k]/T)
                z = work.tile([128, G, F, NB], F32, tag="z")
                x_b = x_sb[:, :, :].unsqueeze(3).broadcast_to((128, G, F, NB))
                wt_b = (
                    wt_sb[:, :]
                    .rearrange("p (f k) -> p f k", k=NB)
                    .unsqueeze(1)
                    .broadcast_to((128, G, F, NB))
                )
                bt_b = (
                    bt_sb[:, :]
                    .rearrange("p (f k) -> p f k", k=NB)
                    .unsqueeze(1)
                    .broadcast_to((128, G, F, NB))
                )
                nc.gpsimd.tensor_mul(z[:, :, :, :], x_b, wt_b)
                nc.gpsimd.tensor_add(z[:, :, :, :], z[:, :, :, :], bt_b)

                # sigma = sigmoid(z)   [128, G, 24]
                sig = work.tile([128, G, F * NB], F32, tag="sig")
                nc.scalar.activation(
                    sig[:, :, :].rearrange("p g (f k) -> p g f k", k=NB),
                    z[:, :, :, :],
                    mybir.ActivationFunctionType.Sigmoid,
                )

                def _kron16(dst, c0, c1, eng=nc.vector):
                    in0 = (
                        sig[:, :, c0:c0 + NB]
                        .unsqueeze(3)
                        .broadcast_to((128, G, NB, NB))
                    )
                    in1 = (
                        sig[:, :, c1:c1 + NB]
                        .unsqueeze(2)
                        .broadcast_to((128, G, NB, NB))
                    )
                    eng.tensor_mul(dst, in0, in1)

                def _pair_cols(c0, tag, eng=nc.vector):
                    # sp[p,g, j*2+t] = sig[p,g, c0+j]  (duplicated pairs)
                    spdt = F16 if ACC_MODE == "fast" else F32
                    sp = work.tile([128, G, NB, 2], spdt, tag=tag)
                    eng.tensor_copy(
                        sp[:, :, :, :],
                        sig[:, :, c0:c0 + NB].unsqueeze(3)
                           .broadcast_to((128, G, NB, 2)),
                    )
                    return sp

                def _kron16_paired(dst, c0, sp, eng=nc.vector):
                    # dst[p,g,i,(j,t)] = s[c0+i] * sp[(j,t)] -- 3 free dims
                    in0 = (
                        sig[:, :, c0:c0 + NB]
                        .unsqueeze(3)
                        .broadcast_to((128, G, NB, 2 * NB))
                    )
                    in1 = (
                        sp[:, :, :, :].rearrange("p g j t -> p g (j t)")
                        .unsqueeze(2)
                        .broadcast_to((128, G, NB, 2 * NB))
                    )
                    eng.tensor_mul(dst, in0, in1)

                if ACC_MODE == "acc":
                    a_sb = work.tile([128, G, 16], F32, tag="A")
                    _kron16(a_sb[:, :, :].rearrange("p g (i j) -> p g i j", j=NB), 0, 4)
                else:
                    # A duplicated x2: ap2[p,g, a*2+t] = s0[k0]*s1[k1], a=k0*4+k1
                    s1p = _pair_cols(4, "s1p")
                    ap2 = work.tile([128, G, 16, 2], F16, tag="A")
                    _kron16_paired(
                        ap2[:, :, :, :].rearrange("p g (i j) t -> p g i (j t)", j=NB), 0, s1p)
                u_sb = work.tile([128, G, 16], F16, tag="u")
                _kron16(u_sb[:, :, :].rearrange("p g (i j) -> p g i j", j=NB), 8, 12,
                        eng=nc.gpsimd)
                # v duplicated x2: vp2[p,g, vv*2+t] = s4[k4]*s5[k5], vv=k4*4+k5
                s5p = _pair_cols(20, "s5p", eng=nc.gpsimd)
                vp2 = work.tile([128, G, 16, 2], F16, tag="v")
                _kron16_paired(
                    vp2[:, :, :, :].rearrange("p g (i j) t -> p g i (j t)", j=NB), 16, s5p,
                    eng=nc.gpsimd)

                # Bm[p,g, vv*16+uu] = u[p,g,uu] * v[p,g,vv]   [128, G, 256]
                # (column order vv-major; folded into the host L3 layout).
                # Per-q ops keep APs at 3 free dims; innermost [1,2] fp16
                # pairs put the DVE in its 2x mode.
                bm = work.tile([128, G, 256], F16, tag="bm")
                for q in range(G):
                    nc.vector.tensor_mul(
                        bm[:, q, :].rearrange("p (i j t) -> p i j t", j=8, t=2),
                        u_sb[:, q, :].rearrange("p (j t) -> p j t", t=2)
                            .unsqueeze(1).broadcast_to((128, 16, 8, 2)),
                        vp2[:, q, :, :].unsqueeze(2).broadcast_to((128, 16, 8, 2)),
                    )

                # M[p, q, a*10+c] accumulated over the 256-contraction.
                # Two half-supertile PSUM tiles so matmuls of the next group
                # can start while the previous group is being copied out.
                if ACC_MODE != "acc":
                    msb = work.tile([128, G, 160], F16, tag="msb")
                else:
                    prod32 = work.tile([128, G, 160], F32, tag="msb")
                for m in range(G // 4):
                    tp = ps_t.tile([128, 4, 256], F16, tag="tp")
                    bmt4 = wts.tile([128, 4, 256], F16, tag="bmt")
                    for qq in range(4):
                        q = m * 4 + qq
                        nc.tensor.transpose(
                            tp[:, qq, 0:128], bm[:, q, 0:128], id_sb[:, :])
                        nc.tensor.transpose(
                            tp[:, qq, 128:256], bm[:, q, 128:256], id_sb[:, :])
                    # one batched evacuation per 4 row-groups; uint32 bitcast
                    # halves the element count
                    nc.scalar.copy(
                        bmt4[:, :, :].bitcast(mybir.dt.uint32),
                        tp[:, :, :].bitcast(mybir.dt.uint32),
                    )
                    mps = ps_m.tile([128, 4, 256], F32, tag="m")
                    for qq in range(4):
                        q = m * 4 + qq
                        nc.tensor.matmul(
                            mps[:, qq, 0:160], bmt4[:, qq, 0:128], l3_sb[:, 0, :],
                            start=True, stop=False,
                        )
                        nc.tensor.matmul(
                            mps[:, qq, 0:160], bmt4[:, qq, 128:256], l3_sb[:, 1, :],
                            start=False, stop=True,
                        )
                    if ACC_MODE != "acc":
                        # M -> SBUF fp16 (scalar engine)
                        nc.scalar.copy(
                            msb[:, m * 4:(m + 1) * 4, :], mps[:, :, 0:160])
                    else:
                        # prod in fp32 straight from PSUM
                        nc.vector.tensor_mul(
                            prod32[:, m * 4:(m + 1) * 4, :]
                                .rearrange("p g (a c) -> p g a c", c=NCLS),
                            a_sb[:, m * 4:(m + 1) * 4, :].unsqueeze(3)
                                .broadcast_to((128, 4, 16, NCLS)),
                            mps[:, :, 0:160].rearrange("p g (a c) -> p g a c", c=NCLS),
                        )

                oq = io.tile([128, G, NCLS], F32, tag="oq")
                if ACC_MODE == "acc":
                    nc.vector.tensor_reduce(
                        oq[:, :, :],
                        prod32[:, :, :].rearrange("p g (a c) -> p g c a", c=NCLS),
                        axis=mybir.AxisListType.X,
                        op=mybir.AluOpType.add,
                    )
                else:
                    # prod[p,g, a*10+c] = A[a] * M[a*10+c]; packed pairs -> 2x
                    prod = work.tile([128, G, 160], F16, tag="prod")
                    for q in range(G):
                        nc.vector.tensor_mul(
                            prod[:, q, :].rearrange("p (a cp t) -> p a cp t", cp=5, t=2),
                            ap2[:, q, :, :].unsqueeze(2).broadcast_to((128, 16, 5, 2)),
                            msb[:, q, :].rearrange("p (a cp t) -> p a cp t", cp=5, t=2),
                        )
                    if ACC_MODE == "mid":
                        nc.vector.tensor_reduce(
                            oq[:, :, :],
                            prod[:, :, :].rearrange("p g (a c) -> p g c a", c=NCLS),
                            axis=mybir.AxisListType.X,
                            op=mybir.AluOpType.add,
                        )
                    else:
                        f1 = work.tile([128, G, 80], F16, tag="f1")
                        nc.vector.tensor_add(f1[:, :, :], prod[:, :, 0:80], prod[:, :, 80:160])
                        f2 = work.tile([128, G, 40], F16, tag="f2")
                        nc.vector.tensor_add(f2[:, :, :], f1[:, :, 0:40], f1[:, :, 40:80])
                        nc.vector.tensor_reduce(
                            oq[:, :, :],
                            f2[:, :, :].rearrange("p g (a c) -> p g c a", c=NCLS),
                            axis=mybir.AxisListType.X,
                            op=mybir.AluOpType.add,
                        )

                od = outc[base:base + ST_ROWS, :].rearrange("(p g) c -> p g c", g=G)
                nc.sync.dma_start(od, oq[:, :, :])

    nc.compile()
    return nc


def _host_prep(x, beta, leaves2classes):
    x = np.ascontiguousarray(np.asarray(x, dtype=np.float32))
    beta = np.asarray(beta, dtype=np.float32)
    L = np.asarray(leaves2classes, dtype=np.float32)

    w = np.linspace(1.0, float(NB), NB, dtype=np.float32)
    bs = np.sort(beta)
    b = np.concatenate([np.zeros(1, np.float32), np.cumsum(-bs, dtype=np.float32)])

    wt24 = np.tile(w / np.float32(TEMP), F).astype(np.float32)       # [(f,k)] = w[k]/T
    bt24 = np.tile(b / np.float32(TEMP), F).astype(np.float32)
    WT = np.ascontiguousarray(np.broadcast_to(wt24, (128, 24)))
    BT = np.ascontiguousarray(np.broadcast_to(bt24, (128, 24)))

    # L3[b, a*10+c] = L[a*256+b, c];  then rows permuted to the device's
    # Bm column order j = vv*16+uu  (b_leaf = uu*16+vv)
    L3 = L.reshape(16, 256, NCLS).transpose(1, 0, 2).reshape(256, 16 * NCLS)
    j = np.arange(256)
    L3 = L3[(j % 16) * 16 + (j // 16)]
    L3P = np.ascontiguousarray(L3.reshape(2, 128, 16 * NCLS).transpose(1, 0, 2)).astype(np.float16)

    ident = np.eye(128, dtype=np.float16)
    return x, WT, BT, ident, L3P


def kernel(x, beta, leaves2classes):
    x, WT, BT, ident, L3P = _host_prep(x, beta, leaves2classes)

    if "nc" not in _NC_CACHE:
        _NC_CACHE["nc"] = _build_nc()
    nc = _NC_CACHE["nc"]

    in_maps = []
    for c in range(CORES):
        in_maps.append({
            "xc": np.ascontiguousarray(x[c * ROWS:(c + 1) * ROWS]),
            "wt": WT,
            "bt": BT,
            "ident": ident,
            "l3p": L3P,
        })
    res = run_bass_kernel_spmd(nc, in_maps, core_ids=list(range(CORES)))
    out = np.concatenate([r["outc"] for r in res.results], axis=0)
    return out.astype(np.float32)



# revision 2
# speedup vs baseline: 1.4049x; 1.4049x over previous
"""DNDT forward kernel for Trainium2 (8 NeuronCores, data-parallel).

Math (matches the reference):
    w = [1,2,3,4];  b = [0, cumsum(-sort(beta))]
    sigma[i,f,k] = sigmoid((x[i,f]*w[k] + b[k]) / T)            [B, 6, 4]
    leaves[i]    = kron(sigma[i,0], ..., sigma[i,5])            [B, 4096]
    out          = leaves @ L                                   [B, 10]

Restructured to avoid materializing the 4096-wide leaves:
    A[i,a]  = kron(s0, s1)          a = k0*4+k1      in [0,16)
    Bm[i,b] = kron(s2, s3, s4, s5)  b in [0,256)
    M[i,(c,a)] = sum_b Bm[i,b] * L3[b, (c,a)]   (PE matmul, K=256 in 2 chunks)
    out[i,c]   = sum_a A[i,a] * M[i,(c,a)]      (DVE multiply + pairwise adds)

Device layout tricks:
  - z = (x*w+b)/T precomputed on host, shipped f16 in supertile layout,
    loaded in a single DMA.
  - Bm columns are ordered so that consecutive f16 pairs belong to the two
    K-chunks of the contraction (uu-parity split).  One PE transpose of the
    f32-bitcast [128,128] tile then yields BOTH chunks' weights; the
    matmul's LDWEIGHTS reads them with a stride-2 f16 access pattern.
    The L3 row permutation compensating for this order is done on host.
  - M is produced (c,a)-major (host permutes L3 columns), so the final
    A-contraction is a single 3-free-dim fp16 tensor_tensor at DVE 2x mode
    (innermost `a` is stride-1 for both operands, no duplication needed),
    followed by log2(16) pairwise adds.
  - prod/tree/out for supertile k are emitted one iteration behind the
    build/matmul stages of supertile k+1 (software pipelining), so the DVE
    never stalls on the PE->scalar evacuation chain.

Per-core: 8192 rows as 8 supertiles of 1024 rows; partition p holds rows
{st*1024 + p*8 + g : g in [0,8)}.
"""

import numpy as np

import concourse.bacc as bacc
import concourse.mybir as mybir
import concourse.tile as tile
from concourse.bass_utils import run_bass_kernel_spmd

F32 = mybir.dt.float32
F16 = mybir.dt.float16

B, F, NB, NCLS = 65536, 6, 4, 10
CORES = 8
ROWS = B // CORES          # 8192 rows per core
G = 8                      # row-groups per supertile
ST_ROWS = 128 * G          # 1024 rows per supertile
N_ST = ROWS // ST_ROWS     # 8 supertiles
TEMP = 0.1

_NC_CACHE = {}


def _build_nc():
    nc = bacc.Bacc("TRN2", target_bir_lowering=False, debug=False)

    zt = nc.dram_tensor("zt", [128, N_ST, G, 24], F16, kind="ExternalInput")
    l3p = nc.dram_tensor("l3p", [128, 2, 160], F16, kind="ExternalInput")
    identf = nc.dram_tensor("identf", [128, 128], F32, kind="ExternalInput")
    outc = nc.dram_tensor("outc", [128, N_ST, G, NCLS], F32, kind="ExternalOutput")

    with tile.TileContext(nc) as tc:
        with (
            tc.tile_pool(name="consts", bufs=1) as consts,
            tc.tile_pool(name="work", bufs=3) as work,
            tc.tile_pool(name="io", bufs=3) as io,
            tc.tile_pool(name="wts", bufs=3) as wts,
            tc.tile_pool(name="ps_t", bufs=2, space="PSUM") as ps_t,
            tc.tile_pool(name="ps_m", bufs=3, space="PSUM") as ps_m,
        ):
            z_sb = consts.tile([128, N_ST, G, 24], F16)
            nc.sync.dma_start(z_sb[:, :, :, :], zt[:, :, :, :])
            l3_sb = consts.tile([128, 2, 160], F16)
            nc.sync.dma_start(l3_sb[:, :, :], l3p[:, :, :])
            id_sb = consts.tile([128, 128], F32)
            nc.sync.dma_start(id_sb[:, :], identf[:, :])

            state = {}

            def stage_C(k):
                # A-contraction for supertile k: prod + pairwise-add tree + DMA
                a_sb, msb = state[k]
                prodc = work.tile([128, G, NCLS, 16], F16, tag="prodc")
                nc.vector.tensor_mul(
                    prodc[:, :, :, :],
                    a_sb[:, :, :].unsqueeze(2).broadcast_to((128, G, NCLS, 16)),
                    msb[:, :, :].rearrange("p g (c a) -> p g c a", a=16),
                )
                f1 = work.tile([128, G, NCLS, 8], F16, tag="f1")
                nc.vector.tensor_add(
                    f1[:, :, :, :], prodc[:, :, :, 0:8], prodc[:, :, :, 8:16])
                f2 = work.tile([128, G, NCLS, 4], F16, tag="f2")
                nc.vector.tensor_add(
                    f2[:, :, :, :], f1[:, :, :, 0:4], f1[:, :, :, 4:8])
                f3 = work.tile([128, G, NCLS, 2], F16, tag="f3")
                nc.vector.tensor_add(
                    f3[:, :, :, :], f2[:, :, :, 0:2], f2[:, :, :, 2:4])
                oq = io.tile([128, G, NCLS], F32, tag="oq")
                nc.vector.tensor_add(oq[:, :, :], f3[:, :, :, 0], f3[:, :, :, 1])
                nc.sync.dma_start(outc[:, k, :, :], oq[:, :, :])

            for k in range(N_ST):
                sig = work.tile([128, G, 24], F16, tag="sig")
                nc.scalar.activation(
                    sig[:, :, :], z_sb[:, k, :, :],
                    mybir.ActivationFunctionType.Sigmoid,
                )

                # kron builds on gpsimd: u = s2 x s3, vp2 = s4 x dup(s5)
                u_sb = work.tile([128, G, 16], F16, tag="u")
                nc.gpsimd.tensor_mul(
                    u_sb[:, :, :].rearrange("p g (i j) -> p g i j", j=NB),
                    sig[:, :, 8:12].unsqueeze(3).broadcast_to((128, G, NB, NB)),
                    sig[:, :, 12:16].unsqueeze(2).broadcast_to((128, G, NB, NB)),
                )
                s5p = work.tile([128, G, NB, 2], F16, tag="s5p")
                nc.gpsimd.tensor_copy(
                    s5p[:, :, :, :],
                    sig[:, :, 20:24].unsqueeze(3).broadcast_to((128, G, NB, 2)),
                )
                vp2 = work.tile([128, G, 16, 2], F16, tag="vp2")
                nc.gpsimd.tensor_mul(
                    vp2[:, :, :, :].rearrange("p g (i j) t -> p g i (j t)", j=NB),
                    sig[:, :, 16:20].unsqueeze(3).broadcast_to((128, G, NB, 2 * NB)),
                    s5p[:, :, :, :].rearrange("p g j t -> p g (j t)")
                       .unsqueeze(2).broadcast_to((128, G, NB, 2 * NB)),
                )

                # previous supertile's tail runs while gpsimd builds this one
                if k > 0:
                    stage_C(k - 1)

                # A = s0 x s1 (plain, no duplication)   [128, G, 16]
                a_sb = work.tile([128, G, 16], F16, tag="A")
                nc.vector.tensor_mul(
                    a_sb[:, :, :].rearrange("p g (i j) -> p g i j", j=NB),
                    sig[:, :, 0:4].unsqueeze(3).broadcast_to((128, G, NB, NB)),
                    sig[:, :, 4:8].unsqueeze(2).broadcast_to((128, G, NB, NB)),
                )

                # Bm[p,g, vv*16 + 2*j + t] = u[p,g,2j+t] * v[p,g,vv]
                # one fused 4-free-dim fp16 op, 2x mode
                bm = work.tile([128, G, 256], F16, tag="bm")
                nc.vector.tensor_mul(
                    bm[:, :, :].rearrange("p g (i j t) -> p g i j t", j=8, t=2),
                    u_sb[:, :, :].rearrange("p g (j t) -> p g j t", t=2)
                        .unsqueeze(2).broadcast_to((128, G, 16, 8, 2)),
                    vp2[:, :, :, :].unsqueeze(3).broadcast_to((128, G, 16, 8, 2)),
                )

                msb = work.tile([128, G, 160], F16, tag="msb")
                for m in range(G // 4):
                    tp = ps_t.tile([128, 4, 128], F32, tag="tp")
                    for qq in range(4):
                        q = m * 4 + qq
                        nc.tensor.transpose(
                            tp[:, qq, :], bm[:, q, :].bitcast(F32), id_sb[:, :])
                    bmt4 = wts.tile([128, 4, 128], F32, tag="bmt")
                    nc.scalar.copy(bmt4[:, :, :], tp[:, :, :])
                    mps = ps_m.tile([128, 4, 256], F32, tag="m")
                    for qq in range(4):
                        w16 = bmt4[:, qq, :].bitcast(F16).rearrange(
                            "p (j t) -> p t j", t=2)
                        nc.tensor.matmul(
                            mps[:, qq, 0:160], w16[:, 0, :], l3_sb[:, 0, :],
                            start=True, stop=False,
                        )
                        nc.tensor.matmul(
                            mps[:, qq, 0:160], w16[:, 1, :], l3_sb[:, 1, :],
                            start=False, stop=True,
                        )
                    nc.scalar.copy(
                        msb[:, m * 4:(m + 1) * 4, :], mps[:, :, 0:160])

                state[k] = (a_sb, msb)

            stage_C(N_ST - 1)

    nc.compile()
    return nc


def _host_prep(x, beta, leaves2classes):
    x = np.asarray(x, dtype=np.float32)
    beta = np.asarray(beta, dtype=np.float32)
    L = np.asarray(leaves2classes, dtype=np.float32)

    w = np.linspace(1.0, float(NB), NB, dtype=np.float32)
    bs = np.sort(beta)
    b = np.concatenate([np.zeros(1, np.float32), np.cumsum(-bs, dtype=np.float32)])

    # z[i, f*4+k] = (x[i,f]*w[k] + b[k]) / T
    z = (x[:, :, None] * w[None, None, :] + b[None, None, :]) / np.float32(TEMP)
    z = z.reshape(B, F * NB).astype(np.float16)

    # L3r[b_leaf, a, c] = L[a*256 + b_leaf, c]
    L3r = L.reshape(16, 256, NCLS).transpose(1, 0, 2)      # [256, 16, 10]
    # device Bm column order: col = vv*16 + uu, packed pair (2J, 2J+1):
    #   J = vv*8 + uu//2, T = uu%2;  b_leaf = uu*16 + vv
    J = np.arange(128)
    l3p = np.empty((128, 2, 160), np.float32)
    for T in range(2):
        uu = 2 * (J % 8) + T
        vv = J // 8
        rows = L3r[uu * 16 + vv]                           # [128, 16, 10]
        # columns (c, a)-major: col = c*16 + a
        l3p[:, T, :] = rows.transpose(0, 2, 1).reshape(128, 160)
    l3p = l3p.astype(np.float16)

    ident = np.eye(128, dtype=np.float32)
    return z, l3p, ident


def _prep_in_maps(x, beta, leaves2classes):
    z, l3p, ident = _host_prep(x, beta, leaves2classes)
    in_maps = []
    for c in range(CORES):
        zc = z[c * ROWS:(c + 1) * ROWS].reshape(N_ST, 128, G, 24)
        zc = np.ascontiguousarray(zc.transpose(1, 0, 2, 3))
        in_maps.append({"zt": zc, "l3p": l3p, "identf": ident})
    return in_maps


def kernel(x, beta, leaves2classes):
    in_maps = _prep_in_maps(x, beta, leaves2classes)

    if "nc" not in _NC_CACHE:
        _NC_CACHE["nc"] = _build_nc()
    nc = _NC_CACHE["nc"]

    res = run_bass_kernel_spmd(nc, in_maps, core_ids=list(range(CORES)))
    outs = []
    for r in res.results:
        oc = r["outc"]                                     # [128, N_ST, G, 10]
        outs.append(oc.transpose(1, 0, 2, 3).reshape(ROWS, NCLS))
    return np.concatenate(outs, axis=0).astype(np.float32)


# revision 5
# speedup vs baseline: 1.4751x; 1.0500x over previous
"""DNDT forward kernel for Trainium2 (8 NeuronCores, data-parallel).

Math (matches the reference):
    w = [1,2,3,4];  b = [0, cumsum(-sort(beta))]
    sigma[i,f,k] = sigmoid((x[i,f]*w[k] + b[k]) / T)            [B, 6, 4]
    leaves[i]    = kron(sigma[i,0], ..., sigma[i,5])            [B, 4096]
    out          = leaves @ L                                   [B, 10]

Restructured to avoid materializing the 4096-wide leaves:
    A[i,a]  = kron(s0, s1)          a = k0*4+k1      in [0,16)
    Bm[i,b] = kron(s2, s3, s4, s5)  b in [0,256)
    M[i,(c,a)] = sum_b Bm[i,b] * L3[b, (c,a)]   (PE matmul, K=256 in 2 chunks)
    out[i,c]   = sum_a A[i,a] * M[i,(c,a)]      (DVE multiply + pairwise adds)

Device tricks:
  - z = (x*w+b)/T precomputed on host, shipped f16 in supertile layout.
  - Bm columns ordered so consecutive f16 pairs belong to the two K-chunks
    (uu-parity).  One PE transpose of the f32-bitcast [128,128] tile yields
    both chunks' weights; LDWEIGHTS reads them with a stride-2 f16 AP.
    The compensating L3 row permutation is done on host.
  - M is (c,a)-major (host permutes L3 columns) so the A-contraction is a
    3-free-dim fp16 tensor_tensor at DVE 2x (stride-1 innermost for both
    operands, no duplication), followed by pairwise adds.
  - Cheap stages (sigmoid, krons, A, prod, add-tree) are fused across
    supertile blocks [0],[1,2],[3,4],[5,6],[7] to amortize per-op engine
    overhead; bm + transpose + matmul stay per-supertile for pipelining.
  - Supertile 0's krons run on the (otherwise idle at ramp) DVE; later
    blocks' krons run on GpSimd one block ahead of use.
  - PSUM->SBUF evacuation of M is split: m-group 0 on Scalar, 1 on GpSimd.
  - prod/tree/output for block b are emitted after block b+1's matmuls
    (software pipelining), so the DVE never waits on the PE/Scalar chain.

Per-core: 8192 rows as 8 supertiles of 1024 rows; partition p holds rows
{st*1024 + p*8 + g : g in [0,8)}.
"""

import numpy as np

import concourse.bacc as bacc
import concourse.mybir as mybir
import concourse.tile as tile
from concourse.bass_utils import run_bass_kernel_spmd

F32 = mybir.dt.float32
F16 = mybir.dt.float16

B, F, NB, NCLS = 65536, 6, 4, 10
CORES = 8
ROWS = B // CORES          # 8192 rows per core
G = 8                      # row-groups per supertile
ST_ROWS = 128 * G          # 1024 rows per supertile
N_ST = ROWS // ST_ROWS     # 8 supertiles
TEMP = 0.1

BLOCKS = [[0], [1, 2], [3, 4], [5, 6], [7]]

_NC_CACHE = {}


def _build_nc():
    nc = bacc.Bacc("TRN2", target_bir_lowering=False, debug=False)

    zt = nc.dram_tensor("zt", [128, N_ST, G, 24], F16, kind="ExternalInput")
    l3p = nc.dram_tensor("l3p", [128, 2, 160], F16, kind="ExternalInput")
    identf = nc.dram_tensor("identf", [128, 128], F32, kind="ExternalInput")
    outc = nc.dram_tensor("outc", [128, N_ST, G, NCLS], F32, kind="ExternalOutput")

    with tile.TileContext(nc) as tc:
        with (
            tc.tile_pool(name="big", bufs=1) as big,
            tc.tile_pool(name="work", bufs=3) as work,
            tc.tile_pool(name="io", bufs=2) as io,
            tc.tile_pool(name="wts", bufs=3) as wts,
            tc.tile_pool(name="ps_t", bufs=2, space="PSUM") as ps_t,
            tc.tile_pool(name="ps_m", bufs=2, space="PSUM") as ps_m,
        ):
            # singly-written resident tensors
            z_sb = big.tile([128, N_ST, G, 24], F16)
            sig = big.tile([128, N_ST, G, 24], F16)
            u_t = big.tile([128, N_ST, G, 16], F16)
            s5p = big.tile([128, N_ST, G, NB, 2], F16)
            vp2 = big.tile([128, N_ST, G, 16, 2], F16)
            a_t = big.tile([128, N_ST, G, 16], F16)
            msb = big.tile([128, N_ST, G, 160], F16)
            l3_sb = big.tile([128, 2, 160], F16)
            id_sb = big.tile([128, 128], F32)

            # input DMAs: first block's z, then consts, then the rest
            nc.sync.dma_start(z_sb[:, 0:1], zt[:, 0:1])
            nc.sync.dma_start(id_sb[:, :], identf[:, :])
            nc.sync.dma_start(l3_sb[:, :, :], l3p[:, :, :])
            nc.sync.dma_start(z_sb[:, 1:3], zt[:, 1:3])
            nc.sync.dma_start(z_sb[:, 3:5], zt[:, 3:5])
            nc.sync.dma_start(z_sb[:, 5:7], zt[:, 5:7])
            nc.sync.dma_start(z_sb[:, 7:8], zt[:, 7:8])

            def blk_view(t, blk, tail):
                # [128, n*G] + tail view of a big tensor's block slice
                n = len(blk)
                return t[:, blk[0]:blk[0] + n].rearrange(
                    f"p n g {tail} -> p (n g) {tail}")

            def sigmoid(blk):
                k0, n = blk[0], len(blk)
                nc.scalar.activation(
                    sig[:, k0:k0 + n], z_sb[:, k0:k0 + n],
                    mybir.ActivationFunctionType.Sigmoid,
                )

            def krons(blk, eng):
                # u = s2 x s3;  vp2 = s4 x dup(s5)   (fused over block rows)
                ng = len(blk) * G
                sg = blk_view(sig, blk, "c")            # [128, ng, 24]
                eng.tensor_mul(
                    blk_view(u_t, blk, "a").rearrange(
                        "p s (i j) -> p s i j", j=NB),
                    sg[:, :, 8:12].unsqueeze(3).broadcast_to((128, ng, NB, NB)),
                    sg[:, :, 12:16].unsqueeze(2).broadcast_to((128, ng, NB, NB)),
                )
                eng.tensor_copy(
                    blk_view(s5p, blk, "j t"),
                    sg[:, :, 20:24].unsqueeze(3).broadcast_to((128, ng, NB, 2)),
                )
                eng.tensor_mul(
                    blk_view(vp2, blk, "v t").rearrange(
                        "p s (i j) t -> p s i (j t)", j=NB),
                    sg[:, :, 16:20].unsqueeze(3).broadcast_to(
                        (128, ng, NB, 2 * NB)),
                    blk_view(s5p, blk, "j t").rearrange(
                        "p s j t -> p s (j t)").unsqueeze(2).broadcast_to(
                        (128, ng, NB, 2 * NB)),
                )

            def a_kron(blk):
                ng = len(blk) * G
                sg = blk_view(sig, blk, "c")
                nc.vector.tensor_mul(
                    blk_view(a_t, blk, "a").rearrange(
                        "p s (i j) -> p s i j", j=NB),
                    sg[:, :, 0:4].unsqueeze(3).broadcast_to((128, ng, NB, NB)),
                    sg[:, :, 4:8].unsqueeze(2).broadcast_to((128, ng, NB, NB)),
                )

            def stage_C(blk):
                # prod + pairwise-add tree + output DMA for a block
                k0, n = blk[0], len(blk)
                ng = n * G
                prodc = work.tile([128, ng, NCLS, 16], F16, tag="prodc")
                nc.vector.tensor_mul(
                    prodc[:, :, :, :],
                    blk_view(a_t, blk, "a").unsqueeze(2).broadcast_to(
                        (128, ng, NCLS, 16)),
                    blk_view(msb, blk, "ca").rearrange(
                        "p s (c a) -> p s c a", a=16),
                )
                f1 = work.tile([128, ng, NCLS, 8], F16, tag="f1")
                nc.vector.tensor_add(
                    f1[:, :, :, :], prodc[:, :, :, 0:8], prodc[:, :, :, 8:16])
                f2 = work.tile([128, ng, NCLS, 4], F16, tag="f2")
                nc.vector.tensor_add(
                    f2[:, :, :, :], f1[:, :, :, 0:4], f1[:, :, :, 4:8])
                f3 = work.tile([128, ng, NCLS, 2], F16, tag="f3")
                nc.vector.tensor_add(
                    f3[:, :, :, :], f2[:, :, :, 0:2], f2[:, :, :, 2:4])
                oq = io.tile([128, ng, NCLS], F32, tag="oq")
                nc.vector.tensor_add(oq[:, :, :], f3[:, :, :, 0], f3[:, :, :, 1])
                nc.sync.dma_start(
                    outc[:, k0:k0 + n],
                    oq[:, :, :].rearrange("p (n g) c -> p n g c", n=n))

            def stage_B(k):
                # transposes + matmuls + M evacuation for supertile k
                bm = state[k]
                for m in range(2):
                    tp = ps_t.tile([128, 4, 128], F32, tag="tp")
                    for qq in range(4):
                        q = m * 4 + qq
                        nc.tensor.transpose(
                            tp[:, qq, :], bm[:, q, :].bitcast(F32), id_sb[:, :])
                    bmt4 = wts.tile([128, 4, 128], F32, tag="bmt")
                    nc.scalar.copy(bmt4[:, :, :], tp[:, :, :])
                    mps = ps_m.tile([128, 4, 256], F32, tag="m")
                    for qq in range(4):
                        w16 = bmt4[:, qq, :].bitcast(F16).rearrange(
                            "p (j t) -> p t j", t=2)
                        nc.tensor.matmul(
                            mps[:, qq, 0:160], w16[:, 0, :], l3_sb[:, 0, :],
                            start=True, stop=False,
                        )
                        nc.tensor.matmul(
                            mps[:, qq, 0:160], w16[:, 1, :], l3_sb[:, 1, :],
                            start=False, stop=True,
                        )
                    nc.scalar.copy(
                        msb[:, k, m * 4:(m + 1) * 4, :], mps[:, :, 0:160])

            state = {}

            # sigmoids for every block up front (scalar)
            for blk in BLOCKS:
                sigmoid(blk)
            # block 0 krons on the ramp-idle DVE; block 1 on gpsimd
            krons(BLOCKS[0], nc.vector)
            krons(BLOCKS[1], nc.gpsimd)

            for bi, blk in enumerate(BLOCKS):
                if bi + 2 < len(BLOCKS):
                    krons(BLOCKS[bi + 2], nc.gpsimd)
                a_kron(blk)
                for k in blk:
                    # Bm[p,g, vv*16 + 2j + t] = u[p,g,2j+t] * v[p,g,vv]
                    bm = work.tile([128, G, 256], F16, tag="bm")
                    nc.vector.tensor_mul(
                        bm[:, :, :].rearrange(
                            "p g (i j t) -> p g i j t", j=8, t=2),
                        u_t[:, k].rearrange("p g (j t) -> p g j t", t=2)
                            .unsqueeze(2).broadcast_to((128, G, 16, 8, 2)),
                        vp2[:, k].unsqueeze(3).broadcast_to((128, G, 16, 8, 2)),
                    )
                    state[k] = bm
                    stage_B(k)
                if bi > 0:
                    stage_C(BLOCKS[bi - 1])
            stage_C(BLOCKS[-1])

    nc.compile()
    return nc


def _host_prep(x, beta, leaves2classes):
    x = np.asarray(x, dtype=np.float32)
    beta = np.asarray(beta, dtype=np.float32)
    L = np.asarray(leaves2classes, dtype=np.float32)

    w = np.linspace(1.0, float(NB), NB, dtype=np.float32)
    bs = np.sort(beta)
    b = np.concatenate([np.zeros(1, np.float32), np.cumsum(-bs, dtype=np.float32)])

    # z[i, f*4+k] = (x[i,f]*w[k] + b[k]) / T
    z = (x[:, :, None] * w[None, None, :] + b[None, None, :]) / np.float32(TEMP)
    z = z.reshape(B, F * NB).astype(np.float16)

    # L3r[b_leaf, a, c] = L[a*256 + b_leaf, c]
    L3r = L.reshape(16, 256, NCLS).transpose(1, 0, 2)      # [256, 16, 10]
    # device Bm column order: col = vv*16 + uu, packed pair (2J, 2J+1):
    #   J = vv*8 + uu//2, T = uu%2;  b_leaf = uu*16 + vv
    J = np.arange(128)
    l3p = np.empty((128, 2, 160), np.float32)
    for T in range(2):
        uu = 2 * (J % 8) + T
        vv = J // 8
        rows = L3r[uu * 16 + vv]                           # [128, 16, 10]
        # columns (c, a)-major: col = c*16 + a
        l3p[:, T, :] = rows.transpose(0, 2, 1).reshape(128, 160)
    l3p = l3p.astype(np.float16)

    ident = np.eye(128, dtype=np.float32)
    return z, l3p, ident


def _prep_in_maps(x, beta, leaves2classes):
    z, l3p, ident = _host_prep(x, beta, leaves2classes)
    in_maps = []
    for c in range(CORES):
        zc = z[c * ROWS:(c + 1) * ROWS].reshape(N_ST, 128, G, 24)
        zc = np.ascontiguousarray(zc.transpose(1, 0, 2, 3))
        in_maps.append({"zt": zc, "l3p": l3p, "identf": ident})
    return in_maps


def kernel(x, beta, leaves2classes):
    in_maps = _prep_in_maps(x, beta, leaves2classes)

    if "nc" not in _NC_CACHE:
        _NC_CACHE["nc"] = _build_nc()
    nc = _NC_CACHE["nc"]

    res = run_bass_kernel_spmd(nc, in_maps, core_ids=list(range(CORES)))
    outs = []
    for r in res.results:
        oc = r["outc"]                                     # [128, N_ST, G, 10]
        outs.append(oc.transpose(1, 0, 2, 3).reshape(ROWS, NCLS))
    return np.concatenate(outs, axis=0).astype(np.float32)


# revision 11
# speedup vs baseline: 1.5143x; 1.0266x over previous
"""DNDT forward kernel for Trainium2 (8 NeuronCores, data-parallel).

Math (matches the reference):
    w = [1,2,3,4];  b = [0, cumsum(-sort(beta))]
    sigma[i,f,k] = sigmoid((x[i,f]*w[k] + b[k]) / T)            [B, 6, 4]
    leaves[i]    = kron(sigma[i,0], ..., sigma[i,5])            [B, 4096]
    out          = leaves @ L                                   [B, 10]

Restructured to avoid materializing the 4096-wide leaves:
    A[i,a]  = kron(s0, s1)          a = k0*4+k1      in [0,16)
    Bm[i,b] = kron(s2, s3, s4, s5)  b in [0,256)
    M[i,(c,a)] = sum_b Bm[i,b] * L3[b, (c,a)]   (PE matmul, K=256 in 2 chunks)
    out[i,c]   = sum_a A[i,a] * M[i,(c,a)]      (DVE multiply + pairwise adds)

Device tricks:
  - z = (x*w+b)/T precomputed on host, shipped f16 in supertile layout.
  - Bm columns ordered so consecutive f16 pairs belong to the two K-chunks
    (uu-parity).  One PE transpose of the f32-bitcast [128,128] tile yields
    both chunks' weights; LDWEIGHTS reads them with a stride-2 f16 AP.
    The compensating L3 row permutation is done on host.
  - M is (c,a)-major (host permutes L3 columns) so the A-contraction is a
    3-free-dim fp16 tensor_tensor at DVE 2x (stride-1 innermost for both
    operands, no duplication), followed by pairwise adds.
  - Cheap stages (sigmoid, krons, A, prod, add-tree) are fused across
    supertile blocks [0],[1,2],[3,4],[5,6],[7] to amortize per-op engine
    overhead; bm + transpose + matmul stay per-supertile for pipelining.
  - Supertile 0's krons run on the (otherwise idle at ramp) DVE; later
    blocks' krons run on GpSimd one block ahead of use.
  - PSUM->SBUF evacuation of M is split: m-group 0 on Scalar, 1 on GpSimd.
  - prod/tree/output for block b are emitted after block b+1's matmuls
    (software pipelining), so the DVE never waits on the PE/Scalar chain.

Per-core: 8192 rows as 8 supertiles of 1024 rows; partition p holds rows
{st*1024 + p*8 + g : g in [0,8)}.
"""

import numpy as np

import concourse.bacc as bacc
import concourse.mybir as mybir
import concourse.tile as tile
from concourse.bass_utils import run_bass_kernel_spmd

F32 = mybir.dt.float32
F16 = mybir.dt.float16

B, F, NB, NCLS = 65536, 6, 4, 10
CORES = 8
ROWS = B // CORES          # 8192 rows per core
G = 8                      # row-groups per supertile
ST_ROWS = 128 * G          # 1024 rows per supertile
N_ST = ROWS // ST_ROWS     # 8 supertiles
TEMP = 0.1

BLOCKS = [[0], [1, 2], [3, 4], [5, 6], [7]]

_NC_CACHE = {}


def _build_nc():
    nc = bacc.Bacc("TRN2", target_bir_lowering=False, debug=False)

    zt = nc.dram_tensor("zt", [128, N_ST, G, 24], F16, kind="ExternalInput")
    l3p = nc.dram_tensor("l3p", [128, 2, 160], F16, kind="ExternalInput")
    identf = nc.dram_tensor("identf", [128, 128], F32, kind="ExternalInput")
    outc = nc.dram_tensor("outc", [128, N_ST, G, NCLS], F32, kind="ExternalOutput")

    with tile.TileContext(nc) as tc:
        with (
            tc.tile_pool(name="big", bufs=1) as big,
            tc.tile_pool(name="work", bufs=3) as work,
            tc.tile_pool(name="io", bufs=2) as io,
            tc.tile_pool(name="wts", bufs=3) as wts,
            tc.tile_pool(name="ps_t", bufs=2, space="PSUM") as ps_t,
            tc.tile_pool(name="ps_m", bufs=2, space="PSUM") as ps_m,
        ):
            # singly-written resident tensors
            z_sb = big.tile([128, N_ST, G, 24], F16)
            sig = big.tile([128, N_ST, G, 24], F16)
            u_t = big.tile([128, N_ST, G, 16], F16)
            s5p = big.tile([128, N_ST, G, NB, 2], F16)
            vp2 = big.tile([128, N_ST, G, 16, 2], F16)
            a_t = big.tile([128, N_ST, G, 16], F16)
            msb = big.tile([128, N_ST, G, 160], F16)
            l3_sb = big.tile([128, 2, 160], F16)
            id_sb = big.tile([128, 128], F32)

            # input DMAs: z for the first three supertiles, then consts
            # (needed ~2us later), then the rest of z
            nc.sync.dma_start(z_sb[:, 0:1], zt[:, 0:1])
            nc.sync.dma_start(z_sb[:, 1:3], zt[:, 1:3])
            nc.sync.dma_start(id_sb[:, :], identf[:, :])
            nc.sync.dma_start(l3_sb[:, :, :], l3p[:, :, :])
            nc.sync.dma_start(z_sb[:, 3:5], zt[:, 3:5])
            nc.sync.dma_start(z_sb[:, 5:7], zt[:, 5:7])
            nc.sync.dma_start(z_sb[:, 7:8], zt[:, 7:8])

            def blk_view(t, blk, tail):
                # [128, n*G] + tail view of a big tensor's block slice
                n = len(blk)
                return t[:, blk[0]:blk[0] + n].rearrange(
                    f"p n g {tail} -> p (n g) {tail}")

            def sigmoid(blk):
                k0, n = blk[0], len(blk)
                nc.scalar.activation(
                    sig[:, k0:k0 + n], z_sb[:, k0:k0 + n],
                    mybir.ActivationFunctionType.Sigmoid,
                )

            def krons(blk, eng):
                # u = s2 x s3;  vp2[.., vv, t] = s4[k4]*s5[k5] (dup via
                # stride-0 reads, no separate dup op)
                ng = len(blk) * G
                sg = blk_view(sig, blk, "c")            # [128, ng, 24]
                eng.tensor_mul(
                    blk_view(u_t, blk, "a").rearrange(
                        "p s (i j) -> p s i j", j=NB),
                    sg[:, :, 8:12].unsqueeze(3).broadcast_to((128, ng, NB, NB)),
                    sg[:, :, 12:16].unsqueeze(2).broadcast_to((128, ng, NB, NB)),
                )
                nc.vector.tensor_copy(
                    blk_view(s5p, blk, "j t"),
                    sg[:, :, 20:24].unsqueeze(3).broadcast_to((128, ng, NB, 2)),
                )
                eng.tensor_mul(
                    blk_view(vp2, blk, "v t").rearrange(
                        "p s (i j) t -> p s i (j t)", j=NB),
                    sg[:, :, 16:20].unsqueeze(3).broadcast_to(
                        (128, ng, NB, 2 * NB)),
                    blk_view(s5p, blk, "j t").rearrange(
                        "p s j t -> p s (j t)").unsqueeze(2).broadcast_to(
                        (128, ng, NB, 2 * NB)),
                )

            def a_kron(blk, eng):
                ng = len(blk) * G
                sg = blk_view(sig, blk, "c")
                eng.tensor_mul(
                    blk_view(a_t, blk, "a").rearrange(
                        "p s (i j) -> p s i j", j=NB),
                    sg[:, :, 0:4].unsqueeze(3).broadcast_to((128, ng, NB, NB)),
                    sg[:, :, 4:8].unsqueeze(2).broadcast_to((128, ng, NB, NB)),
                )

            def stage_C(blk):
                # prod + pairwise-add tree + output DMA for a block
                k0, n = blk[0], len(blk)
                ng = n * G
                prodc = work.tile([128, ng, NCLS, 16], F16, tag="prodc")
                nc.vector.tensor_mul(
                    prodc[:, :, :, :],
                    blk_view(a_t, blk, "a").unsqueeze(2).broadcast_to(
                        (128, ng, NCLS, 16)),
                    blk_view(msb, blk, "ca").rearrange(
                        "p s (c a) -> p s c a", a=16),
                )
                f1 = work.tile([128, ng, NCLS, 8], F16, tag="f1")
                nc.vector.tensor_add(
                    f1[:, :, :, :], prodc[:, :, :, 0:8], prodc[:, :, :, 8:16])
                f2 = work.tile([128, ng, NCLS, 4], F16, tag="f2")
                nc.vector.tensor_add(
                    f2[:, :, :, :], f1[:, :, :, 0:4], f1[:, :, :, 4:8])
                f3 = work.tile([128, ng, NCLS, 2], F16, tag="f3")
                nc.vector.tensor_add(
                    f3[:, :, :, :], f2[:, :, :, 0:2], f2[:, :, :, 2:4])
                oq = io.tile([128, ng, NCLS], F32, tag="oq")
                nc.vector.tensor_add(oq[:, :, :], f3[:, :, :, 0], f3[:, :, :, 1])
                nc.sync.dma_start(
                    outc[:, k0:k0 + n],
                    oq[:, :, :].rearrange("p (n g) c -> p n g c", n=n))

            def stage_B(k):
                # transposes + matmuls + M evacuation for supertile k
                bm = state[k]
                for m in range(2):
                    tp = ps_t.tile([128, 4, 128], F32, tag="tp")
                    for qq in range(4):
                        q = m * 4 + qq
                        nc.tensor.transpose(
                            tp[:, qq, :], bm[:, q, :].bitcast(F32), id_sb[:, :])
                    bmt4 = wts.tile([128, 4, 128], F32, tag="bmt")
                    nc.scalar.copy(bmt4[:, :, :], tp[:, :, :])
                    mps = ps_m.tile([128, 4, 256], F32, tag="m")
                    for qq in range(4):
                        w16 = bmt4[:, qq, :].bitcast(F16).rearrange(
                            "p (j t) -> p t j", t=2)
                        nc.tensor.matmul(
                            mps[:, qq, 0:160], w16[:, 0, :], l3_sb[:, 0, :],
                            start=True, stop=False,
                        )
                        nc.tensor.matmul(
                            mps[:, qq, 0:160], w16[:, 1, :], l3_sb[:, 1, :],
                            start=False, stop=True,
                        )
                    nc.scalar.copy(
                        msb[:, k, m * 4:(m + 1) * 4, :], mps[:, :, 0:160])

            state = {}

            # sigmoids for every block up front (scalar)
            for blk in BLOCKS:
                sigmoid(blk)
            # block 0 krons on the ramp-idle DVE; block 1 + all A on gpsimd
            krons(BLOCKS[0], nc.vector)
            a_kron(BLOCKS[0], nc.gpsimd)
            krons(BLOCKS[1], nc.gpsimd)
            a_kron(BLOCKS[1], nc.gpsimd)

            for bi, blk in enumerate(BLOCKS):
                if bi + 2 < len(BLOCKS):
                    krons(BLOCKS[bi + 2], nc.gpsimd)
                    a_kron(BLOCKS[bi + 2], nc.gpsimd)
                for k in blk:
                    # Bm[p,g, vv*16 + 2j + t] = u[p,g,2j+t] * v[p,g,vv]
                    bm = work.tile([128, G, 256], F16, tag="bm")
                    nc.vector.tensor_mul(
                        bm[:, :, :].rearrange(
                            "p g (i j t) -> p g i j t", j=8, t=2),
                        u_t[:, k].rearrange("p g (j t) -> p g j t", t=2)
                            .unsqueeze(2).broadcast_to((128, G, 16, 8, 2)),
                        vp2[:, k].unsqueeze(3).broadcast_to((128, G, 16, 8, 2)),
                    )
                    state[k] = bm
                    stage_B(k)
                if bi > 0:
                    stage_C(BLOCKS[bi - 1])
            stage_C(BLOCKS[-1])

    nc.compile()
    return nc


def _host_prep(x, beta, leaves2classes):
    x = np.asarray(x, dtype=np.float32)
    beta = np.asarray(beta, dtype=np.float32)
    L = np.asarray(leaves2classes, dtype=np.float32)

    w = np.linspace(1.0, float(NB), NB, dtype=np.float32)
    bs = np.sort(beta)
    b = np.concatenate([np.zeros(1, np.float32), np.cumsum(-bs, dtype=np.float32)])

    # z[i, f*4+k] = (x[i,f]*w[k] + b[k]) / T
    z = (x[:, :, None] * w[None, None, :] + b[None, None, :]) / np.float32(TEMP)
    z = z.reshape(B, F * NB).astype(np.float16)

    # L3r[b_leaf, a, c] = L[a*256 + b_leaf, c]
    L3r = L.reshape(16, 256, NCLS).transpose(1, 0, 2)      # [256, 16, 10]
    # device Bm column order: col = vv*16 + uu, packed pair (2J, 2J+1):
    #   J = vv*8 + uu//2, T = uu%2;  b_leaf = uu*16 + vv
    J = np.arange(128)
    l3p = np.empty((128, 2, 160), np.float32)
    for T in range(2):
        uu = 2 * (J % 8) + T
        vv = J // 8
        rows = L3r[uu * 16 + vv]                           # [128, 16, 10]
        # columns (c, a)-major: col = c*16 + a
        l3p[:, T, :] = rows.transpose(0, 2, 1).reshape(128, 160)
    l3p = l3p.astype(np.float16)

    ident = np.eye(128, dtype=np.float32)
    return z, l3p, ident


def _prep_in_maps(x, beta, leaves2classes):
    z, l3p, ident = _host_prep(x, beta, leaves2classes)
    in_maps = []
    for c in range(CORES):
        zc = z[c * ROWS:(c + 1) * ROWS].reshape(N_ST, 128, G, 24)
        zc = np.ascontiguousarray(zc.transpose(1, 0, 2, 3))
        in_maps.append({"zt": zc, "l3p": l3p, "identf": ident})
    return in_maps


def kernel(x, beta, leaves2classes):
    in_maps = _prep_in_maps(x, beta, leaves2classes)

    if "nc" not in _NC_CACHE:
        _NC_CACHE["nc"] = _build_nc()
    nc = _NC_CACHE["nc"]

    res = run_bass_kernel_spmd(nc, in_maps, core_ids=list(range(CORES)))
    outs = []
    for r in res.results:
        oc = r["outc"]                                     # [128, N_ST, G, 10]
        outs.append(oc.transpose(1, 0, 2, 3).reshape(ROWS, NCLS))
    return np.concatenate(outs, axis=0).astype(np.float32)
